# revision 1
# baseline (speedup 1.0000x reference)
"""CrossAttention kernel for 8 Trainium2 NeuronCores.

Problem (hardcoded shapes): B=4, N=1024, C=1024, E=1024, H=16, D=64.
  kv = x @ Wkv + bkv ; k, v = split(kv) ; q = query @ Wq + bq
  keys = [k; q] (2N), values = [v; v]
  out = softmax(q keys^T / sqrt(D)) @ values        -> [B, N, E]

Sharding: 8 cores = 4 batches x 2 head-groups (8 heads each).
Per-core strategy (all matmul contractions run on the partition dim):
  - x^T, query^T shipped host-transposed [C, N]
  - q^T, k^T computed on-chip as [Ecol, N] (head-pair-major partitions)
  - scores computed transposed [keys, queries] so the PV matmul needs no
    on-chip transposes; both query blocks share one 2-bank PSUM tile so
    exp runs 1024 wide; softmax denominator comes from a ones-column
    appended to the V stationary; output returned transposed [Ecol, N]
    and un-transposed on the host.
All matmuls run in float32r (full PE rate at fp32 storage, ~1e-4 rel err).
"""
import numpy as np

B, N, C, E, H = 4, 1024, 1024, 1024, 16
D = E // H            # 64
HPC = 8               # heads per core
EC = HPC * D          # 512 E-columns per core
NCORES = 8
CT = C // 128         # 8 contraction tiles
ST = N // 128         # 8 seq tiles
KT = 2 * N // 128     # 16 key tiles (k then q-as-keys)
PAIRS = HPC // 2      # 4 head pairs

_compiled = None


def _build():
    import concourse.bass as bass
    import concourse.bacc as bacc
    import concourse.mybir as mybir
    import concourse.tile as tile
    import contextlib

    F32 = mybir.dt.float32
    F32R = mybir.dt.float32r
    EXP = mybir.ActivationFunctionType.Exp

    nc = bacc.Bacc()
    xT_in = nc.declare_dram_parameter("xT", [C, N], F32R, isOutput=False)
    qryT_in = nc.declare_dram_parameter("qryT", [C, N], F32R, isOutput=False)
    wq_in = nc.declare_dram_parameter("wq", [C, EC], F32R, isOutput=False)
    wk_in = nc.declare_dram_parameter("wk", [C, EC], F32R, isOutput=False)
    wv_in = nc.declare_dram_parameter("wv", [C, EC], F32R, isOutput=False)
    bq_in = nc.declare_dram_parameter("bq", [EC], F32R, isOutput=False)
    bk_in = nc.declare_dram_parameter("bk", [EC], F32R, isOutput=False)
    bv_in = nc.declare_dram_parameter("bv", [EC], F32R, isOutput=False)
    ones_in = nc.declare_dram_parameter("ones", [512], F32R, isOutput=False)
    out_o = nc.declare_dram_parameter("out_t", [EC, N], F32, isOutput=True)

    with tile.TileContext(nc) as tc, contextlib.ExitStack() as ctx:
        pers = ctx.enter_context(tc.tile_pool(name="pers", bufs=1))
        epool = ctx.enter_context(tc.tile_pool(name="epool", bufs=4))
        outp = ctx.enter_context(tc.tile_pool(name="outp", bufs=2))

        # ---- persistent SBUF ----
        xTs = pers.tile([128, CT, N], F32R, tag="xTs")
        qryTs = pers.tile([128, CT, N], F32R, tag="qryTs")
        wqs = pers.tile([128, CT, EC], F32R, tag="wqs")
        wks = pers.tile([128, CT, EC], F32R, tag="wks")
        wvs = pers.tile([128, CT, EC], F32R, tag="wvs")
        qTs = pers.tile([128, PAIRS, N], F32R, tag="qTs")
        kTs = pers.tile([128, PAIRS, N], F32R, tag="kTs")
        vvs = pers.tile([128, ST, HPC, D + 1], F32R, tag="vvs")
        bqr = pers.tile([1, EC], F32R, tag="bqr")
        bkr = pers.tile([1, EC], F32R, tag="bkr")
        bvr = pers.tile([1, EC], F32R, tag="bvr")
        onesr = pers.tile([1, 512], F32R, tag="onesr")

        # ---- loads (SWDGE cast-DMA f32 -> f32r), interleaved in consumption
        # order so the projections start as soon as their first tiles land
        for ct in range(CT):
            nc.sync.dma_start(out=wvs[:, ct, :], in_=wv_in[ct * 128:(ct + 1) * 128, :])
            nc.sync.dma_start(out=xTs[:, ct, :], in_=xT_in[ct * 128:(ct + 1) * 128, :])
            if ct == 0:
                nc.sync.dma_start(out=bqr[:], in_=bq_in.ap().rearrange("(o e) -> o e", o=1))
                nc.sync.dma_start(out=bkr[:], in_=bk_in.ap().rearrange("(o e) -> o e", o=1))
                nc.sync.dma_start(out=bvr[:], in_=bv_in.ap().rearrange("(o e) -> o e", o=1))
                nc.sync.dma_start(out=onesr[:], in_=ones_in.ap().rearrange("(o e) -> o e", o=1))
                _o = ones_in.ap()
                ones_bc = bass.AP(tensor=_o.tensor, offset=_o.offset,
                                  ap=[[0, 128], [ST, HPC], [1, ST], [1, 1]])
                nc.sync.dma_start(out=vvs[:, :, :, D:D + 1], in_=ones_bc)
        for ct in range(CT):
            nc.sync.dma_start(out=wks[:, ct, :], in_=wk_in[ct * 128:(ct + 1) * 128, :])
        for ct in range(CT):
            nc.sync.dma_start(out=wqs[:, ct, :], in_=wq_in[ct * 128:(ct + 1) * 128, :])
            nc.sync.dma_start(out=qryTs[:, ct, :], in_=qryT_in[ct * 128:(ct + 1) * 128, :])

        # ================= phase 1: projections =================
        # ct-outer loops with 8 psum accumulators; weights streamed per ct.
        with tc.tile_pool(name="proj_ps", bufs=8, space="PSUM") as proj_ps:
            # V: v[st] = (x @ Wv + bv), natural [seq, Ecol]
            vps = []
            for st in range(ST):
                vp = proj_ps.tile([128, EC], F32, tag="proj")
                vps.append(vp)
            for ct in range(CT):
                for st in range(ST):
                    nc.tensor.matmul(vps[st][:], xTs[:, ct, st * 128:(st + 1) * 128],
                                     wvs[:, ct, :], start=(ct == 0), stop=False)
            for st in range(ST):
                nc.tensor.matmul(vps[st][:], onesr[:, 0:128], bvr[:], start=False, stop=True)
                nc.vector.tensor_copy(out=vvs[:, st, :, 0:D],
                                      in_=vps[st][:].rearrange("p (h d) -> p h d", h=HPC))
            # q^T / k^T: [Ecol(128), N] per head pair, all pairs at once
            for wsb, srcs, brow, dst in ((wks, xTs, bkr, kTs), (wqs, qryTs, bqr, qTs)):
                pps = []
                for i in range(2 * PAIRS):
                    pp = proj_ps.tile([128, 512], F32, tag="proj")
                    pps.append(pp)
                for ct in range(CT):
                    for pair in range(PAIRS):
                        for qb in range(2):
                            nc.tensor.matmul(pps[2 * pair + qb][:],
                                             wsb[:, ct, pair * 128:(pair + 1) * 128],
                                             srcs[:, ct, qb * 512:(qb + 1) * 512],
                                             start=(ct == 0), stop=False)
                for pair in range(PAIRS):
                    pcols = slice(pair * 128, (pair + 1) * 128)
                    for qb in range(2):
                        pp = pps[2 * pair + qb]
                        nc.tensor.matmul(pp[:], brow[:, pcols], onesr[:],
                                         start=False, stop=True)
                        nc.vector.tensor_copy(out=dst[:, pair, qb * 512:(qb + 1) * 512],
                                              in_=pp[:])

        # ================= phase 2: attention =================
        with tc.tile_pool(name="sc_ps", bufs=2, space="PSUM") as sc_ps, \
             tc.tile_pool(name="pv_ps", bufs=4, space="PSUM") as pv_ps:
            for pair in range(PAIRS):
                pcols = slice(pair * 128, (pair + 1) * 128)
                hA, hB = 2 * pair, 2 * pair + 1
                # 4 accumulators: (head A/B) x (query block 0/1)
                pv = [[None, None], [None, None]]
                for hi in range(2):
                    for qb in range(2):
                        pvt = pv_ps.tile([D + 1, 512], F32, tag="pv")
                        pv[hi][qb] = pvt
                # software-pipelined: scores/exp for kt run ahead of PV for kt-1
                # so the ACT engine (the bottleneck) never waits on the PE.
                prev_e2 = None
                for kt in range(KT):
                    src = kTs if kt < ST else qTs
                    ksl = slice((kt % ST) * 128, (kt % ST + 1) * 128)
                    e2 = []
                    for hi, rows in ((0, slice(0, 64)), (1, slice(64, 128))):
                        s2 = sc_ps.tile([128, 1024], F32, tag="sc")
                        nc.tensor.matmul(s2[:, 0:512], src[rows, pair, ksl],
                                         qTs[rows, pair, 0:512])
                        nc.tensor.matmul(s2[:, 512:1024], src[rows, pair, ksl],
                                         qTs[rows, pair, 512:1024])
                        e = epool.tile([128, 1024], F32R, tag="e")
                        nc.scalar.activation(out=e[:], in_=s2[:], func=EXP, scale=0.125)
                        e2.append(e)
                    if prev_e2 is not None:
                        for hi, h in ((0, hA), (1, hB)):
                            for qb in range(2):
                                nc.tensor.matmul(pv[hi][qb][:], vvs[:, (kt - 1) % ST, h, :],
                                                 prev_e2[hi][:, qb * 512:(qb + 1) * 512],
                                                 start=(kt == 1), stop=False)
                    prev_e2 = e2
                for hi, h in ((0, hA), (1, hB)):
                    for qb in range(2):
                        nc.tensor.matmul(pv[hi][qb][:], vvs[:, (KT - 1) % ST, h, :],
                                         prev_e2[hi][:, qb * 512:(qb + 1) * 512],
                                         start=False, stop=True)
                # normalize: out = pv[0:64] / pv[64]; assemble per-qb [128, 512]
                for qb in range(2):
                    osb = outp.tile([128, 512], F32, tag="osb")
                    for hi in range(2):
                        p = pv[hi][qb]
                        rc = outp.tile([1, 512], F32, tag="rc")
                        nc.vector.reciprocal(out=rc[:], in_=p[D:D + 1, :])
                        bc = outp.tile([64, 512], F32, tag="bc")
                        nc.gpsimd.partition_broadcast(bc[:], rc[:], channels=64)
                        nc.vector.tensor_mul(out=osb[hi * 64:(hi + 1) * 64, :],
                                             in0=p[0:D, :], in1=bc[:])
                    nc.sync.dma_start(out=out_o[pcols, qb * 512:(qb + 1) * 512], in_=osb[:])

    nc.finalize()
    return nc


def _get_compiled():
    global _compiled
    if _compiled is None:
        _compiled = _build()
    return _compiled


def kernel(x, query, Wkv, bkv, Wq, bq):
    from concourse.bass_utils import run_bass_kernel_spmd

    x = np.asarray(x, dtype=np.float32)
    query = np.asarray(query, dtype=np.float32)
    Wkv = np.asarray(Wkv, dtype=np.float32)
    bkv = np.asarray(bkv, dtype=np.float32)
    Wq = np.asarray(Wq, dtype=np.float32)
    bq = np.asarray(bq, dtype=np.float32)

    ones = np.ones((512,), np.float32)
    in_maps = []
    for core in range(NCORES):
        b, hg = core // 2, core % 2
        ecs = slice(hg * EC, (hg + 1) * EC)
        in_maps.append({
            "xT": np.ascontiguousarray(x[b].T),
            "qryT": np.ascontiguousarray(query[b].T),
            "wq": np.ascontiguousarray(Wq[:, ecs]),
            "wk": np.ascontiguousarray(Wkv[:, hg * EC:(hg + 1) * EC]),
            "wv": np.ascontiguousarray(Wkv[:, E + hg * EC:E + (hg + 1) * EC]),
            "bq": np.ascontiguousarray(bq[ecs]),
            "bk": np.ascontiguousarray(bkv[hg * EC:(hg + 1) * EC]),
            "bv": np.ascontiguousarray(bkv[E + hg * EC:E + (hg + 1) * EC]),
            "ones": ones,
        })

    nc = _get_compiled()
    res = None
    last_err = None
    for attempt in range(3):
        try:
            res = run_bass_kernel_spmd(nc, in_maps, list(range(NCORES)))
            break
        except Exception as ex:  # transient NRT_EXEC_UNIT_UNRECOVERABLE etc.
            last_err = ex
    if res is None:
        raise last_err

    out = np.empty((B, N, E), np.float32)
    for core in range(NCORES):
        b, hg = core // 2, core % 2
        out[b, :, hg * EC:(hg + 1) * EC] = res.results[core]["out_t"].T
    return out



# revision 8
# speedup vs baseline: 1.2813x; 1.2813x over previous
"""CrossAttention kernel for 8 Trainium2 NeuronCores.

Problem (hardcoded shapes): B=4, N=1024, C=1024, E=1024, H=16, D=64.
  kv = x @ Wkv + bkv ; k, v = split(kv) ; q = query @ Wq + bq
  keys = [k; q] (2N), values = [v; v]
  out = softmax(q keys^T / sqrt(D)) @ values        -> [B, N, E]

Sharding: 8 cores = 4 batches x 2 head-groups (8 heads each).

Per-core design (ACT-exp is the roofline; everything else hides under it):
  - bf16 inputs/weights (host-cast, host-pretiled to SBUF layouts).
  - attention runs per head-pair; the q-as-keys half of the key range runs
    FIRST so exp starts as soon as q^T is projected (x may still be loading).
  - values are duplicated across the two key halves, so PV contracts over
    probs1+probs2 (one bf16 DVE add per tile) - half the PV matmul work.
  - PV orientation out[q, d]: stationary = summed probs [keys,128q] bf16,
    moving = v-tile [keys, 65] bf16 (65th col = ones -> softmax denominator).
    16 accumulators/pair packed 7/7/2 into 3 PSUM banks (73-elem slots so no
    matmul output crosses a bank).
  - scores psum double-buffered (4 banks), next-pair projections accumulate
    in 1 rotating bank; 4+3+1 = 8 banks exactly.
  - biases folded into the PSUM->SBUF copies on DVE (per-partition scalar for
    k/q, broadcast row for v); normalization = reciprocal of the denominator
    column + per-partition scalar multiply on DVE; output stored [N, EC]
    directly (no host transpose).
"""
import numpy as np

B, N, C, E, H = 4, 1024, 1024, 1024, 16
D = E // H            # 64
HPC = 8               # heads per core
EC = HPC * D          # 512 E-columns per core
NCORES = 8
CT = C // 128         # 8 contraction tiles
ST = N // 128         # 8 seq tiles
PAIRS = HPC // 2      # 4 head pairs
KB = N // 128         # 8 key blocks per key half

_compiled = None


def _build():
    import concourse.bass as bass
    import concourse.bacc as bacc
    import concourse.mybir as mybir
    import concourse.tile as tile
    import contextlib
    from collections import deque

    F32 = mybir.dt.float32
    F32R = mybir.dt.float32r
    BF16 = mybir.dt.bfloat16
    EXP = mybir.ActivationFunctionType.Exp
    ADD = mybir.AluOpType.add
    MULT = mybir.AluOpType.mult

    nc = bacc.Bacc()
    xT_in = nc.declare_dram_parameter("xT", [128, CT, N], BF16, isOutput=False)
    qryT_in = nc.declare_dram_parameter("qryT", [128, CT, N], BF16, isOutput=False)
    wq_in = nc.declare_dram_parameter("wq", [128, PAIRS, CT, 128], BF16, isOutput=False)
    wk_in = nc.declare_dram_parameter("wk", [128, PAIRS, CT, 128], BF16, isOutput=False)
    wv_in = nc.declare_dram_parameter("wv", [128, PAIRS, CT, 128], BF16, isOutput=False)
    bq_in = nc.declare_dram_parameter("bqc", [128, PAIRS], F32, isOutput=False)
    bk_in = nc.declare_dram_parameter("bkc", [128, PAIRS], F32, isOutput=False)
    bv_in = nc.declare_dram_parameter("bvv", [128, EC], BF16, isOutput=False)
    out_o = nc.declare_dram_parameter("out_t", [N, EC], F32, isOutput=True)

    with tile.TileContext(nc) as tc, contextlib.ExitStack() as ctx:
        pers = ctx.enter_context(tc.tile_pool(name="pers", bufs=1))
        ekp = ctx.enter_context(tc.tile_pool(name="ekp", bufs=3))
        esp = ctx.enter_context(tc.tile_pool(name="esp", bufs=3))
        outp = ctx.enter_context(tc.tile_pool(name="outp", bufs=3))
        prj = ctx.enter_context(tc.tile_pool(name="prj", bufs=1, space="PSUM"))
        scp = ctx.enter_context(tc.tile_pool(name="scp", bufs=2, space="PSUM"))
        pvp = ctx.enter_context(tc.tile_pool(name="pvp", bufs=1, space="PSUM"))

        # ---- persistent SBUF ----
        xTs = pers.tile([128, CT, N], BF16, tag="xTs")
        qryTs = pers.tile([128, CT, N], BF16, tag="qryTs")
        wqs = pers.tile([128, PAIRS, CT, 128], BF16, tag="wqs")
        wks = pers.tile([128, PAIRS, CT, 128], BF16, tag="wks")
        wvs = pers.tile([128, PAIRS, CT, 128], BF16, tag="wvs")
        qTs = pers.tile([128, PAIRS, N], BF16, tag="qTs")
        kTs = pers.tile([128, PAIRS, N], BF16, tag="kTs")
        vvs = pers.tile([128, ST, HPC, 66], BF16, tag="vvs")
        bqr = pers.tile([128, PAIRS], F32, tag="bqr")
        bkr = pers.tile([128, PAIRS], F32, tag="bkr")
        bvr = pers.tile([128, EC], BF16, tag="bvr")
        # q-part probs for the current pair (overwritten each pair)
        eqs = pers.tile([128, KB, 2, N], BF16, tag="eqs")

        # ---- loads, priority order (DMA is serial): biases, then the
        # q-projection chain (wq pair0 -> qryT), then k (wk p0 -> xT), v,
        # then remaining pairs' weights.
        nc.sync.dma_start(out=bqr[:], in_=bq_in[:, :])
        nc.sync.dma_start(out=bkr[:], in_=bk_in[:, :])
        nc.sync.dma_start(out=bvr[:], in_=bv_in[:, :])
        nc.sync.dma_start(out=wqs[:, 0], in_=wq_in[:, 0])
        for ct in range(CT):
            nc.sync.dma_start(out=qryTs[:, ct], in_=qryT_in[:, ct])
        nc.sync.dma_start(out=wks[:, 0], in_=wk_in[:, 0])
        for ct in range(CT):
            nc.sync.dma_start(out=xTs[:, ct], in_=xT_in[:, ct])
        nc.sync.dma_start(out=wvs[:, 0], in_=wv_in[:, 0])
        for p in range(1, PAIRS):
            nc.sync.dma_start(out=wqs[:, p], in_=wq_in[:, p])
            nc.sync.dma_start(out=wks[:, p], in_=wk_in[:, p])
            nc.sync.dma_start(out=wvs[:, p], in_=wv_in[:, p])

        nc.vector.memset(vvs[:, :, :, 64:65], 1.0)

        # ---- projection emitters (yield every couple of matmuls so they can
        # be pumped into the PE stream between attention steps) ----
        def gen_1_proj(p, wsb, src, dstT, brow):
            for half in range(2):
                hsl = slice(half * 512, (half + 1) * 512)
                pt = prj.tile([128, 512], F32, tag="prj")
                for ct in range(CT):
                    nc.tensor.matmul(pt[:], wsb[:, p, ct, :],
                                     src[:, ct, hsl],
                                     start=(ct == 0), stop=(ct == CT - 1))
                    if ct % 2 == 1:
                        yield
                nc.vector.tensor_scalar(out=dstT[:, p, hsl], in0=pt[:],
                                        scalar1=brow[:, p:p + 1],
                                        scalar2=None, op0=ADD)
                yield

        def gen_q_proj(p):
            yield from gen_1_proj(p, wqs, qryTs, qTs, bqr)

        def gen_k_proj(p):
            yield from gen_1_proj(p, wks, xTs, kTs, bkr)

        def gen_v_proj(p):
            for g in range(2):
                pt = prj.tile([128, 4, 128], F32, tag="prj")
                for ct in range(CT):
                    for si in range(4):
                        st = g * 4 + si
                        # start=True zeroes the WHOLE psum bank: only the
                        # first matmul of the bank starts; siblings accumulate
                        # onto the zeroed bank.
                        nc.tensor.matmul(pt[:, si, :],
                                         xTs[:, ct, st * 128:(st + 1) * 128],
                                         wvs[:, p, ct, :],
                                         start=(ct == 0 and si == 0),
                                         stop=(ct == CT - 1),
                                         skip_group_check=True)
                    yield
                for si in range(4):
                    st = g * 4 + si
                    nc.vector.tensor_add(
                        out=vvs[:, st, 2 * p:2 * p + 2, 0:64],
                        in0=pt[:, si, :].rearrange("q (h d) -> q h d", h=2),
                        in1=bvr[:, p * 128:(p + 1) * 128].rearrange(
                            "q (h d) -> q h d", h=2))
                yield

        bg = deque()

        def pump(n):
            for _ in range(n):
                while bg:
                    try:
                        next(bg[0])
                        break
                    except StopIteration:
                        bg.popleft()
                else:
                    return

        # PV accumulators: 16 accs (hi*8+qc) packed 7/7/2 into 3 banks.
        def acc_of(tiles, j):
            if j < 7:
                return tiles[0], j
            if j < 14:
                return tiles[1], j - 7
            return tiles[2], j - 14

        HI = ((0, slice(0, 64)), (1, slice(64, 128)))

        def emit_scores_exp(p, src, kb, dst):
            ksl = slice(kb * 128, (kb + 1) * 128)
            out = []
            for hi, rows in HI:
                sct = scp.tile([128, N], F32, tag="sc")
                nc.tensor.matmul(sct[:, 0:512], src[rows, p, ksl],
                                 qTs[rows, p, 0:512])
                nc.tensor.matmul(sct[:, 512:1024], src[rows, p, ksl],
                                 qTs[rows, p, 512:1024])
                e = dst(hi)
                nc.scalar.activation(out=e, in_=sct[:], func=EXP, scale=0.125)
                out.append(e)
            return out

        def emit_pv(p, kb, es2, start, stop):
            for hi in range(2):
                for qc in range(8):
                    j = hi * 8 + qc
                    t, jj = acc_of(pv_tiles, j)
                    # start=True zeroes the whole bank; only the first acc of
                    # each of the 3 banks starts (j = 0 / 7 / 14).
                    nc.tensor.matmul(t[:, jj, 0:65],
                                     es2[hi][:, qc * 128:(qc + 1) * 128],
                                     vvs[:, kb, 2 * p + hi, 0:65],
                                     start=(start and j in (0, 7, 14)),
                                     stop=stop, skip_group_check=True)

        # ---- head: only pair 0's q projection runs eagerly (its DMA chain
        # loads first); k/v of pair 0 and everything for later pairs pump
        # through the PE's slack during attention steps.
        for _ in gen_q_proj(0):
            pass
        bg.append(gen_k_proj(0))
        bg.append(gen_v_proj(0))
        for np_ in range(1, PAIRS):
            bg.append(gen_q_proj(np_))
            bg.append(gen_k_proj(np_))
            bg.append(gen_v_proj(np_))

        for p in range(PAIRS):
            # q-as-keys half first: needs only qTs
            for kb in range(KB):
                emit_scores_exp(p, qTs, kb,
                                lambda hi, kb=kb: eqs[:, kb, hi, :])
                pump(3)
            # k half, PV pipelined one step behind
            pv_tiles = (pvp.tile([128, 7, 73], F32, tag="pvA", name="pvA"),
                        pvp.tile([128, 7, 73], F32, tag="pvB", name="pvB"),
                        pvp.tile([128, 2, 73], F32, tag="pvC", name="pvC"))
            pend = None
            for kb in range(KB):
                cur = []

                def dst(hi):
                    e = ekp.tile([128, N], BF16, tag="ek")
                    return e
                es_hi = emit_scores_exp(p, kTs, kb, dst)
                for hi in range(2):
                    est = esp.tile([128, N], BF16, tag="es")
                    nc.vector.tensor_add(out=est[:], in0=es_hi[hi],
                                         in1=eqs[:, kb, hi, :])
                    cur.append(est)
                if pend is not None:
                    emit_pv(p, kb - 1, pend, start=(kb == 1), stop=False)
                pend = cur
                pump(3)
            emit_pv(p, KB - 1, pend, start=False, stop=True)

            # normalize + store [N, EC] directly
            rcps = (outp.tile([128, 7, 1], F32, tag="rA", name="rA"),
                    outp.tile([128, 7, 1], F32, tag="rB", name="rB"),
                    outp.tile([128, 2, 1], F32, tag="rC", name="rC"))
            for t, r in zip(pv_tiles, rcps):
                nc.vector.reciprocal(out=r[:], in_=t[:, :, 64:65])
            for qc in range(8):
                ost = outp.tile([128, 128], F32, tag="osb")
                for hi in range(2):
                    t, jj = acc_of(pv_tiles, hi * 8 + qc)
                    r = rcps[0] if hi * 8 + qc < 7 else (
                        rcps[1] if hi * 8 + qc < 14 else rcps[2])
                    nc.vector.tensor_scalar(
                        out=ost[:, hi * 64:(hi + 1) * 64],
                        in0=t[:, jj, 0:64], scalar1=r[:, jj, :],
                        scalar2=None, op0=MULT)
                nc.sync.dma_start(
                    out=out_o[qc * 128:(qc + 1) * 128,
                              p * 128:(p + 1) * 128],
                    in_=ost[:])

    nc.finalize()
    return nc


def _get_compiled():
    global _compiled
    if _compiled is None:
        _compiled = _build()
    return _compiled


def kernel(x, query, Wkv, bkv, Wq, bq):
    import ml_dtypes
    from concourse.bass_utils import run_bass_kernel_spmd

    bf16 = ml_dtypes.bfloat16
    x = np.asarray(x, dtype=np.float32)
    query = np.asarray(query, dtype=np.float32)
    Wkv = np.asarray(Wkv, dtype=np.float32)
    bkv = np.asarray(bkv, dtype=np.float32)
    Wq = np.asarray(Wq, dtype=np.float32)
    bq = np.asarray(bq, dtype=np.float32)

    def tile_T(a):  # [N, C] -> [128, CT, N] (a.T tiled over contraction)
        return np.ascontiguousarray(
            a.T.reshape(CT, 128, N).transpose(1, 0, 2)).astype(bf16)

    def tile_w(w):  # [C, EC] -> [128, PAIRS, CT, 128]
        return np.ascontiguousarray(
            w.reshape(CT, 128, PAIRS, 128).transpose(1, 2, 0, 3)).astype(bf16)

    in_maps = []
    for core in range(NCORES):
        b, hg = core // 2, core % 2
        ecs = slice(hg * EC, (hg + 1) * EC)
        bv = bkv[E + hg * EC:E + (hg + 1) * EC]
        in_maps.append({
            "xT": tile_T(x[b]),
            "qryT": tile_T(query[b]),
            "wq": tile_w(Wq[:, ecs]),
            "wk": tile_w(Wkv[:, hg * EC:(hg + 1) * EC]),
            "wv": tile_w(Wkv[:, E + hg * EC:E + (hg + 1) * EC]),
            "bqc": np.ascontiguousarray(bq[ecs].reshape(PAIRS, 128).T),
            "bkc": np.ascontiguousarray(
                bkv[hg * EC:(hg + 1) * EC].reshape(PAIRS, 128).T),
            "bvv": np.ascontiguousarray(
                np.tile(bv[None, :], (128, 1)).astype(bf16)),
        })

    nc = _get_compiled()
    res = None
    last_err = None
    for attempt in range(3):
        try:
            res = run_bass_kernel_spmd(nc, in_maps, list(range(NCORES)))
            break
        except Exception as ex:  # transient NRT_EXEC_UNIT_UNRECOVERABLE etc.
            last_err = ex
    if res is None:
        raise last_err

    out = np.empty((B, N, E), np.float32)
    for core in range(NCORES):
        b, hg = core // 2, core % 2
        out[b, :, hg * EC:(hg + 1) * EC] = res.results[core]["out_t"]
    return out
